# revision 50
# baseline (speedup 1.0000x reference)
"""CPC loss kernel for Trainium2, data-parallel over 8 NeuronCores.

Math (v2: pair-sum-of-exponentials — no per-pair exp on device)
----
Per row x of shape [C], target t, y = x[t], E_j = e^{x_j}, C = 128:
  ce  = LSE - y,           LSE = ln(sum_j E_j)
  bdc = (P1f - C*y - ln2)/(C-1),       P1f = sum_{all j} ln(E_j + E_t)
  bec = [2*(LNS - P1f + y + ln2) - (C-2)(S - y) + (C-1)ln2] * c2
        with LNS = sum_{j<k} ln(E_j + E_k),  S = sum_j x_j,
        c2 = 0.5/((C-1)(C-2))
using sp(a-b) + sp(b-a) = 2 ln(e^a + e^b) - a - b and
sp(x_j - y) = ln(E_j + E_t) - y.  Collected:

  row_loss = LSE + K_y*y + K_P1*P1f + K_S*S + 2*c2*LNS + CONST

The key structural win: s_f = E_j(f) + E_k(f) for all 8128 j<k pairs is a
MATMUL of E^T (bf16, host-precomputed) with a constant 0/1 pair-incidence
matrix W2[c, f] = delta(c, j(f)) + delta(c, k(f)).  The old kernel needed a
per-pair ScalarE exp (8192 elems/row-batch, making ScalarE the sole
bottleneck); here the exp count drops to O(C) on the host and the per-pair
transcendental is Ln, which can be split across two engines (see the route
comment above _PATTERNS).  An extra W2 column of ones gives SE = sum_j E_j
per row for free (LSE).

Per-row losses are DMA'd out; the host sums across rows and cores.
"""

import functools

import numpy as np
import ml_dtypes

import concourse.bass as bass
import concourse.tile as tile
import concourse.hw_specs as hw_specs
from concourse import bacc, mybir
from concourse.bass_utils import run_bass_kernel_spmd

# The act-table chooser greedily picks the first set containing each
# function; blank the single-function sets so Exp and Ln both resolve to
# natural_log_exp_and_others and a single table load suffices.
_orig_get_activation_tables = hw_specs.get_activation_tables


@functools.cache
def _patched_activation_tables(module_arch: str):
    d = dict(_orig_get_activation_tables(module_arch))
    for name in ("exp_and_others", "natural_log", "exp_and_friends"):
        if name in d:
            d[name] = set()
    return d


hw_specs.get_activation_tables = _patched_activation_tables
bacc.get_activation_tables = _patched_activation_tables

N, C = 16384, 128
NCORES = 8
ROWS = N // NCORES            # rows per core
P = 128                       # partitions / rows per batch
NB = ROWS // P                # batches per core
NPAIR = (C * (C - 1)) // 2    # 8128
SECOL = NPAIR                 # ones-column (SE) at col 8128
NF = 8192                     # 8128 pairs + SE + 63 dead cols
CHUNK = 1024                  # pair cols per PSUM chunk (2 banks)
NCHUNK = NF // CHUNK          # 8
MM_N = 512                    # moving free dim per matmul (1 PSUM bank)
WLAST = 960                   # pair cols in the last chunk (rest is SE + dead)

F32 = mybir.dt.float32
BF16 = mybir.dt.bfloat16
AF = mybir.ActivationFunctionType
ALU = mybir.AluOpType

LOG2 = float(np.log(2.0))
C2 = 0.5 / ((C - 1) * (C - 2))
K_Y = -1.0 - C / (C - 1) + C * C2
K_P1 = 1.0 / (C - 1) - 2.0 * C2
K_S = -(C - 2) * C2
C_CONST = LOG2 * (-1.0 / (C - 1) + (C + 1) * C2)

# Per-batch routes for the 8 chunks of 1024.  Verifier constraints: GPSIMD
# cannot touch PSUM, and any instruction may read at most ONE non-scalar
# input from PSUM (which kills two-operand product trees on PSUM data).
# Legal consumers that compress a chunk in one pass:
#   'A': ScalarE Ln in-place + accum_out            (1183 ns / chunk)
#   'R': DVE tensor_reduce(op=mult) over [P, e, 8]  (1235 ns / chunk)
#        -> products of 8 consecutive pair-sums, ln'd in the per-batch
#        lnin pass on ScalarE (+107 ns)
# Pool (SBUF-only) carries the P1/bdc side path.  nA=63/nR=65 balances
# ScalarE ~90 vs DVE ~90 with PE at 55.
_PATTERNS = {
    "a4": ("A", "R", "A", "R", "A", "R", "A", "R"),
    "a3": ("R", "A", "R", "A", "R", "A", "R", "R"),
    "a5": ("A", "R", "A", "R", "A", "R", "A", "A"),  # A-heavy endgame
}
_ROUTE_SEQ = ["a4", "a4", "a3", "a4", "a4", "a3", "a4", "a4", "a3", "a4", "a4", "a4", "a4", "a4", "a4", "a5"]

_cache: dict = {}


def _build_program(repeat: int = 1, route_seq=None) -> bass.Bass:
    routes = [_PATTERNS[k] for k in (route_seq or _ROUTE_SEQ)]
    nc = bacc.Bacc("TRN2")

    x_d = nc.declare_dram_parameter("x", [ROWS, C], F32, isOutput=False)
    et_d = nc.declare_dram_parameter("et", [C, ROWS], BF16, isOutput=False)
    eb_d = nc.declare_dram_parameter("eb", [ROWS, C], BF16, isOutput=False)
    w_d = nc.declare_dram_parameter("w2", [C, NF], BF16, isOutput=False)
    io_d = nc.declare_dram_parameter("io", [P, C], F32, isOutput=False)
    tf_d = nc.declare_dram_parameter("tf", [ROWS], F32, isOutput=False)
    out_d = nc.declare_dram_parameter("out", [ROWS], F32, isOutput=True)

    with tile.TileContext(nc) as tc:
        with (
            tc.tile_pool(name="const", bufs=1) as const_pool,
            tc.tile_pool(name="work", bufs=4) as work,
            tc.tile_pool(name="acc", bufs=1) as acc_pool,
            tc.tile_pool(name="psum", bufs=4, space="PSUM") as psum_pool,
        ):
            # DMA order: batch-0 matmul inputs first (et cols 0:128 + w2
            # chunk 0 — ~0.7us) so the first A-chunk Ln starts ~1.2us in;
            # x/eb (y-gather, P1 — off the critical path) stream in last
            et_sb = const_pool.tile([C, ROWS], BF16)
            nc.sync.dma_start(out=et_sb[:, :P], in_=et_d[:, :P])
            w_sb = const_pool.tile([C, NF], BF16)
            nc.sync.dma_start(out=w_sb[:, :CHUNK], in_=w_d[:, :CHUNK])
            nc.sync.dma_start(out=et_sb[:, P:], in_=et_d[:, P:])
            io_sb = const_pool.tile([P, C], F32)
            nc.sync.dma_start(out=io_sb, in_=io_d[:])
            t_sb = const_pool.tile([P, NB], F32)
            nc.sync.dma_start(out=t_sb, in_=tf_d.rearrange("(b p) -> p b", p=P))
            for ch in (2, 1, 4, 6, 3, 5, 7):
                nc.sync.dma_start(
                    out=w_sb[:, ch * CHUNK : (ch + 1) * CHUNK],
                    in_=w_d[:, ch * CHUNK : (ch + 1) * CHUNK],
                )
            x_sb = const_pool.tile([P, NB, C], F32)
            nc.sync.dma_start(out=x_sb, in_=x_d.rearrange("(b p) c -> p b c", p=P))
            eb_sb = const_pool.tile([P, NB, C], BF16)
            nc.sync.dma_start(out=eb_sb, in_=eb_d.rearrange("(b p) c -> p b c", p=P))

            # accumulators; LNSACC slots s*NB+b: s=0..4 A-chunk accums (in
            # per-batch order of occurrence), s=5 the lnin (R routes) accum
            LNSACC = acc_pool.tile([P, 6 * NB], F32)
            Y = acc_pool.tile([P, NB], F32)
            SU = acc_pool.tile([P, NB], F32)
            SEb = acc_pool.tile([P, NB], F32)
            EY = acc_pool.tile([P, NB], F32)
            PADD = acc_pool.tile([P, NB, C], BF16)
            P1T1 = acc_pool.tile([P, NB, C // 2], BF16)
            P1T2 = acc_pool.tile([P, NB, C // 4], BF16)
            P1T3 = acc_pool.tile([P, NB, C // 8], BF16)
            P1SCR = acc_pool.tile([P, NB, C // 8], F32)
            P1F = acc_pool.tile([P, NB], F32)
            LSE = acc_pool.tile([P, NB], F32)
            L = acc_pool.tile([P, NB], F32)

            for _rep in range(repeat):
                nc.vector.memset(LNSACC[:, 3 * NB : 5 * NB], 0.0)
                lnin_pending = []  # deferred per-batch lnin Ln: (lnin, nli, b)

                def _flush_lnin():
                    lnp, nlip, bp = lnin_pending.pop(0)
                    lnscr = work.tile([P, 640], F32, tag="lnscr")
                    nc.scalar.activation(
                        lnscr[:, :nlip], lnp[:, :nlip], AF.Ln, bias=0.0,
                        scale=1.0,
                        accum_out=LNSACC[:, 5 * NB + bp : 5 * NB + bp + 1],
                    )

                # y = x[r, t_r]: (iota == t) * x, summed, one STT per batch.
                # Emitted up front: DVE is idle during the DMA ramp, and the
                # P1 halves below want Y as early as possible.
                for b in range(NB):
                    ymscr = work.tile([P, C], F32, tag="ym")
                    nc.vector.scalar_tensor_tensor(
                        ymscr, io_sb, t_sb[:, b : b + 1], x_sb[:, b, :],
                        op0=ALU.is_equal, op1=ALU.mult,
                        accum_out=Y[:, b : b + 1],
                    )
                # S = sum_j x_j, all batches in one 3D reduce (early: only
                # needs x)
                nc.vector.tensor_reduce(
                    SU, x_sb, axis=mybir.AxisListType.X, op=ALU.add
                )

                # P1f = sum_j ln(E_j + e^y), in two stages per half so the
                # ScalarE Ln never waits on Pool's trees (the exp stage runs
                # two batches earlier, giving Pool time to finish)
                def _p1_exp(h0, h1):
                    nc.scalar.activation(
                        EY[:, h0:h1], Y[:, h0:h1], AF.Exp, bias=0.0, scale=1.0
                    )
                    for b2 in range(h0, h1):
                        nc.gpsimd.tensor_scalar(
                            PADD[:, b2, :], eb_sb[:, b2, :], EY[:, b2 : b2 + 1],
                            None, op0=ALU.add,
                        )
                    nc.gpsimd.tensor_mul(
                        P1T1[:, h0:h1], PADD[:, h0:h1, : C // 2],
                        PADD[:, h0:h1, C // 2 :],
                    )
                    nc.gpsimd.tensor_mul(
                        P1T2[:, h0:h1], P1T1[:, h0:h1, : C // 4],
                        P1T1[:, h0:h1, C // 4 :],
                    )
                    nc.gpsimd.tensor_mul(
                        P1T3[:, h0:h1], P1T2[:, h0:h1, : C // 8],
                        P1T2[:, h0:h1, C // 8 :],
                    )

                def _combine(h0, h1):
                    # row_loss = LSE + K_Y*y + K_P1*P1f + K_S*S
                    #          + 2*C2*LNS + CONST, for batches [h0, h1)
                    sl = slice(h0, h1)
                    nc.scalar.activation(
                        LSE[:, sl], SEb[:, sl], AF.Ln, bias=0.0, scale=1.0
                    )
                    Lh = L[:, sl]
                    nc.vector.tensor_add(
                        Lh, LNSACC[:, h0:h1], LNSACC[:, NB + h0 : NB + h1]
                    )
                    for s5 in range(2, 6):
                        nc.vector.tensor_add(
                            Lh, Lh, LNSACC[:, s5 * NB + h0 : s5 * NB + h1]
                        )
                    nc.vector.scalar_tensor_tensor(
                        Lh, Lh, 2.0 * C2, LSE[:, sl], op0=ALU.mult, op1=ALU.add
                    )
                    nc.vector.scalar_tensor_tensor(
                        Lh, Y[:, sl], K_Y, Lh, op0=ALU.mult, op1=ALU.add
                    )
                    nc.vector.scalar_tensor_tensor(
                        Lh, P1F[:, sl], K_P1, Lh, op0=ALU.mult, op1=ALU.add
                    )
                    nc.vector.scalar_tensor_tensor(
                        Lh, SU[:, sl], K_S, Lh, op0=ALU.mult, op1=ALU.add
                    )
                    nc.vector.tensor_scalar_add(Lh, Lh, C_CONST)

                def _p1_ln(h0, h1):
                    nc.scalar.activation(
                        P1SCR[:, h0:h1], P1T3[:, h0:h1], AF.Ln,
                        bias=0.0, scale=1.0,
                    )
                    nc.vector.tensor_reduce(
                        P1F[:, h0:h1], P1SCR[:, h0:h1],
                        axis=mybir.AxisListType.X, op=ALU.add,
                    )

                for b in range(NB):
                    lhsT = et_sb[:, b * P : (b + 1) * P]
                    lnin = work.tile([P, 640], F32, tag="lnin")
                    nli = 0
                    nslot = 0
                    for ch in range(NCHUNK):
                        route = routes[b][ch]
                        # two PSUM lanes: A-chunks (ScalarE consumer) and
                        # R-chunks (DVE consumer), 2 bufs each so
                        # produce/consume overlaps within each lane
                        pt = psum_pool.tile(
                            [P, CHUNK], F32,
                            tag=("pA" if route == "A" else "pR"), bufs=2,
                        )
                        width = WLAST if ch == NCHUNK - 1 else CHUNK
                        if route == "A":
                            for m in range(CHUNK // MM_N):
                                f0 = ch * CHUNK + m * MM_N
                                nc.tensor.matmul(
                                    pt[:, m * MM_N : (m + 1) * MM_N],
                                    lhsT,
                                    w_sb[:, f0 : f0 + MM_N],
                                )
                            slot = nslot * NB + b
                            nslot += 1
                            nc.scalar.activation(
                                pt[:, :width], pt[:, :width], AF.Ln,
                                bias=0.0, scale=1.0,
                                accum_out=LNSACC[:, slot : slot + 1],
                            )
                        else:
                            for m in range(CHUNK // MM_N):
                                f0 = ch * CHUNK + m * MM_N
                                nc.tensor.matmul(
                                    pt[:, m * MM_N : (m + 1) * MM_N],
                                    lhsT,
                                    w_sb[:, f0 : f0 + MM_N],
                                )
                            # products of 8 consecutive pair-sums in one DVE
                            # reduce (single PSUM input — verifier-legal)
                            e = width // 8
                            nc.vector.tensor_reduce(
                                lnin[:, nli : nli + e],
                                pt[:, :width].rearrange(
                                    "p (g e) -> p g e", e=8
                                ),
                                axis=mybir.AxisListType.X, op=ALU.mult,
                            )
                            nli += e
                        if ch == NCHUNK - 1:
                            # raw SE column (pair cols got Ln'd or tree'd;
                            # the SE col is untouched by either route)
                            nc.vector.tensor_copy(
                                SEb[:, b : b + 1], pt[:, WLAST : WLAST + 1]
                            )
                        if ch == 2 and lnin_pending:
                            # previous batch's lnin Ln, deferred here so it
                            # doesn't block this batch's A-chunks in the
                            # in-order ScalarE queue while it waits on the
                            # previous batch's last tree tails
                            _flush_lnin()
                    lnin_pending.append((lnin, nli, b))
                    if b == 6:
                        _p1_exp(0, NB // 2)
                    elif b == 8:
                        _p1_ln(0, NB // 2)
                    elif b == 10:
                        _p1_exp(NB // 2, NB)
                    elif b == 12:
                        _p1_ln(NB // 2, NB)
                    elif b == 13:
                        _combine(0, NB // 2)
                while lnin_pending:
                    _flush_lnin()
                _combine(NB // 2, NB)

            nc.sync.dma_start(out=out_d.rearrange("(b p) -> p b", p=P), in_=L)

    nc.compile()
    return nc


def _host_constants():
    if "w2" not in _cache:
        ju, ku = np.triu_indices(C, 1)
        w = np.zeros((C, NF), np.float32)
        f = np.arange(NPAIR)
        w[ju, f] = 1.0
        w[ku, f] += 1.0
        w[:, SECOL] = 1.0
        _cache["w2"] = w.astype(ml_dtypes.bfloat16)
        _cache["io"] = np.broadcast_to(
            np.arange(C, dtype=np.float32), (P, C)
        ).copy()
    return _cache["w2"], _cache["io"]


def kernel(inputs: np.ndarray, targets: np.ndarray) -> np.ndarray:
    x = np.ascontiguousarray(np.asarray(inputs, dtype=np.float32))
    t = np.asarray(targets)
    assert x.shape == (N, C) and t.shape == (N,)

    if "nc" not in _cache:
        _cache["nc"] = _build_program()
    nc = _cache["nc"]
    w2, io = _host_constants()

    e = np.exp(x).astype(ml_dtypes.bfloat16)
    et = np.ascontiguousarray(e.T)
    tf = t.astype(np.float32)

    in_maps = []
    for c in range(NCORES):
        r0, r1 = c * ROWS, (c + 1) * ROWS
        in_maps.append(
            {
                "x": np.ascontiguousarray(x[r0:r1]),
                "et": np.ascontiguousarray(et[:, r0:r1]),
                "eb": np.ascontiguousarray(e[r0:r1]),
                "w2": w2,
                "io": io,
                "tf": np.ascontiguousarray(tf[r0:r1]),
            }
        )

    res = run_bass_kernel_spmd(nc, in_maps, list(range(NCORES)))
    total = 0.0
    for c in range(NCORES):
        total += np.sum(res.results[c]["out"].astype(np.float64))
    return np.float32(total / N)
